# revision 1
# baseline (speedup 1.0000x reference)
"""ALIF (adaptive leaky integrate-and-fire) layer on 8 TRN2 NeuronCores.

Reference semantics (per element over time T):
    BatchNorm3d (training stats over T,B,H,W per channel) then
    mem   = beta*mem + xn_t
    spike = (mem - (1 + adapt) > 0)
    mem   = mem * (1 - spike)
    adapt = decay*adapt + gamma*spike

Sharding: channels C=64 are split 8 ways (8 channels per core) so the BN
batch stats are fully core-local (no collectives needed).  Host pre-lays
each core's shard out as [T, 128, 512] where
    partition p = c_local*16 + (b // 2)      (c_local in 0..7, b in 0..31)
    free      f = (b % 2)*256 + h*16 + w
so the per-channel BN affine is a per-partition scale/bias.  Per-channel
stats while the chunked loads stream in: plain sum on ScalarE (Identity +
accum) and sum-of-squares on VectorE (AFFINE_MUL_REDUCE), then a 4-round
stream_shuffle XOR butterfly combines each 16-partition group exactly in
fp32.

The scan carries 2x-scaled state (scaling fp32 by 2 is exact, making the
threshold a single fused compare) and pre-applies the leak to the stored
membrane (Mp := beta * 2*mem), so each step is exactly three VectorE
instructions with the BN normalize folded in (raw x is consumed directly):
    VectorE  P  = (x*scl + biasv) + Mp     AFFINE_THEN_ADD (custom DVE op)
    VectorE  S  = (P - 2) is_gt At         scalar_tensor_tensor -> output
    VectorE  Mp = (S*-beta + beta) * P     AFFINE_MUL_REDUCE (custom DVE op)
    ScalarE  Ad = At * decay
    GpSimd   At' = Ad + S                  (off the VectorE critical path)
Spikes accumulate in per-5-step buffers DMA'd straight out (HWDGE).

Measured on silicon (reps-slope method): ~100 us/exec steady-state,
bit-exact vs the fp32 jax reference (0 of 26.2M spikes differ).
"""

import sys

sys.path.insert(0, "/opt/trn_rl_repo")

import numpy as np

import concourse.bass as bass
import concourse.bacc as bacc
import concourse.tile as tile
from concourse import mybir
from concourse.alu_op_type import AluOpType
from concourse.bass_utils import run_bass_kernel_spmd

BETA = 0.9
DECAY_ADAPT = 0.96
BN_EPS = 1e-4

F32 = mybir.dt.float32
AF = mybir.ActivationFunctionType
AX = mybir.AxisListType

N_CORES = 8
P = 128  # SBUF partitions
GRP = 16  # partitions per channel


def build_alif(T=50, CT=5, FD=512, store_group=1, reps=1, io_lite=False,
               no_scan=False, no_stats=False, SCT=None, deep_bufs=0):
    """Build the single-core Bass graph (SPMD across 8 cores).

    T tiles of [P, FD]; x loaded in chunks of CT tiles.
    """
    if SCT is None:
        SCT = CT
    assert T % CT == 0 and T % SCT == 0
    nchunks = T // CT
    n_per_channel = float(GRP * T * FD)

    nc = bacc.Bacc("TRN2", target_bir_lowering=False, debug=False)
    # io_lite: identical instruction stream and HBM traffic, but all chunk
    # loads/stores alias one CT-sized DRAM region --- tiny kernel IO so the
    # per-call operand staging doesn't mask device time when benchmarking
    xT = CT if io_lite else T
    x_ext = nc.declare_dram_parameter("x", [xT, P, FD], F32, isOutput=False)
    bnw_ext = nc.declare_dram_parameter("bnw", [P, 1], F32, isOutput=False)
    bnb_ext = nc.declare_dram_parameter("bnb", [P, 1], F32, isOutput=False)
    out_ext = nc.declare_dram_parameter("out", [xT, P, FD], F32, isOutput=True)

    with tile.TileContext(nc) as tc:
        with (
            tc.tile_pool(name="xbuf", bufs=1) as xpool,
            tc.tile_pool(name="stats", bufs=1) as stpool,
            tc.tile_pool(name="state", bufs=1) as statepool,
            tc.tile_pool(name="ptile", bufs=2 + int(deep_bufs)) as ppool,
            tc.tile_pool(name="adec", bufs=2 + int(deep_bufs)) as adpool,
            tc.tile_pool(name="stile", bufs=2 + int(deep_bufs)) as spool,
        ):
            for _rep in range(reps):
                # scratch tiles for the small stats intermediates, separated
                # by writing engine so no instruction accumulates too many
                # cross-engine waits (each pool tag pads to a 4 KiB slot)
                scr = stpool.tile([P, 48], F32, tag="scratch")
                col = [0]

                def sv(n=1):
                    c = col[0]
                    col[0] += n
                    return scr[:, c:c + n]

                # warm the ScalarE sqrt table set at t=0 (the sqrt set also
                # carries Copy/Identity filler) so the finalize's real Sqrt
                # doesn't pay the ~2.7us ACT_TABLE_LOAD on the critical path
                warm = sv()
                nc.vector.memset(warm, 1.0)
                wsq = sv()
                nc.scalar.activation(out=wsq, in_=warm, func=AF.Sqrt)

                bnwb = stpool.tile([P, 2], F32, tag="bnwb")
                bnw = bnwb[:, 0:1]
                nc.sync.dma_start(out=bnw, in_=bnw_ext[:, :])
                bnb = bnwb[:, 1:2]
                nc.sync.dma_start(out=bnb, in_=bnb_ext[:, :])

                # ---- load x in chunks; sums chase the DMAs ----
                # plain sum on ScalarE (Identity + accum, fully hidden under
                # the load), sum-of-squares on VectorE via one
                # AFFINE_MUL_REDUCE (x*x + accum) per half-chunk -- cheaper
                # than 512-wide bn_stats and keeps VectorE under the DMA wall
                # (NOT tensor_tensor_reduce: that op crashes the exec unit)
                HC = CT * FD // 2
                s1cols = stpool.tile([P, 2 * nchunks], F32, tag="s1cols")
                s2cols = stpool.tile([P, 2 * nchunks], F32, tag="s2cols")
                dump1 = stpool.tile([P, HC], F32, tag="dump1")
                dump2 = stpool.tile([P, HC], F32, tag="dump2")
                xch = []
                for k in range(nchunks):
                    xk = xpool.tile([P, CT * FD], F32, tag=f"x{k}")
                    nc.sync.dma_start(
                        out=xk[:].rearrange("p (t f) -> p t f", f=FD),
                        in_=x_ext[0 if io_lite else k * CT:
                                  CT if io_lite else (k + 1) * CT, :, :
                                  ].rearrange("t p f -> p t f"),
                    )
                    xch.append(xk)
                    # several instruction structs have a single sync-wait
                    # slot; these probes make VectorE/ScalarE observe each
                    # chunk's DMA semaphore on multi-wait-capable copies so
                    # later ops never carry the DMA wait
                    probe = sv()
                    nc.vector.tensor_copy(probe, xk[:, 0:1])
                    nc.scalar.copy(s1cols[:, 2 * k:2 * k + 1], xk[:, 0:1])
                    if not no_stats:
                        for h in range(2):
                            xh = xk[:, h * HC:(h + 1) * HC]
                            nc.scalar.activation(
                                out=dump1[:], in_=xh, func=AF.Identity,
                                accum_out=s1cols[:, 2 * k + h:2 * k + h + 1],
                            )
                            nc.vector.affine_mul_reduce(
                                out=dump2[:], in0=xh, in1=xh,
                                scale=1.0, bias=0.0,
                                accum_out=s2cols[:, 2 * k + h:2 * k + h + 1],
                            )

                # ---- finalize BN stats -> per-partition scale/bias ----
                if no_stats:
                    scl, biasv = 1.0, 0.0
                else:
                    # per-partition (sum, sumsq), then butterfly all-reduce
                    # within each 16-partition channel group
                    E = sv(2)
                    nc.vector.tensor_reduce(
                        out=E[:, 0:1], in_=s1cols[:], axis=AX.X, op=AluOpType.add
                    )
                    nc.vector.tensor_reduce(
                        out=E[:, 1:2], in_=s2cols[:], axis=AX.X, op=AluOpType.add
                    )
                    cur = E
                    for i, k in enumerate((8, 4, 2, 1)):
                        sh = sv(2)
                        nc.vector.stream_shuffle(sh, cur, [j ^ k for j in range(32)])
                        nxt = sv(2)
                        nc.vector.tensor_tensor(nxt, cur, sh, AluOpType.add)
                        cur = nxt

                    me = sv(2)
                    nc.vector.tensor_scalar_mul(me, cur[:, 0:2], 1.0 / n_per_channel)
                    mean = me[:, 0:1]
                    ex2 = me[:, 1:2]
                    msq = sv()
                    nc.vector.tensor_tensor(msq, mean, mean, AluOpType.mult)
                    var = sv()
                    nc.vector.tensor_tensor(var, ex2, msq, AluOpType.subtract)
                    vpe = sv()
                    nc.vector.tensor_scalar_add(vpe, var, BN_EPS)

                    # rstd = rsqrt(vpe): ACT sqrt seed + DVE reciprocal + 2 Newton steps
                    sq = sv()
                    nc.scalar.activation(out=sq, in_=vpe, func=AF.Sqrt)
                    ya = sv()
                    nc.vector.reciprocal(ya, sq)
                    yy = sv()
                    u = sv()
                    w = sv()
                    yb = sv()
                    yc = sv()
                    ys = [ya, yb, yc]
                    for i in range(2):
                        nc.vector.tensor_tensor(yy, ys[i], ys[i], AluOpType.mult)
                        nc.vector.tensor_tensor(u, yy, vpe, AluOpType.mult)
                        nc.vector.tensor_scalar(
                            out=w, in0=u, scalar1=-0.5, scalar2=1.5,
                            op0=AluOpType.mult, op1=AluOpType.add,
                        )
                        nc.vector.tensor_tensor(ys[i + 1], ys[i], w, AluOpType.mult)
                    rstd = ys[2]

                    # scale = 2*bnw*rstd ; bias = 2*bnb - mean*scale
                    scl = sv()
                    nc.vector.scalar_tensor_tensor(
                        out=scl, in0=bnw, scalar=2.0, in1=rstd,
                        op0=AluOpType.mult, op1=AluOpType.mult,
                    )
                    tmpb = sv()
                    nc.vector.tensor_tensor(tmpb, mean, scl, AluOpType.mult)
                    biasv = sv()
                    nc.vector.scalar_tensor_tensor(
                        out=biasv, in0=bnb, scalar=2.0, in1=tmpb,
                        op0=AluOpType.mult, op1=AluOpType.subtract,
                    )

                if no_scan:
                    nc.sync.dma_start(out=out_ext[0:1, :, 0:4],
                                      in_=x_ext[0:1, :, 0:4])
                    continue
                # ---- the scan ----
                # carried state Mp = beta * 2*mem (leak pre-applied at store),
                # so the custom fused DVE ops cover everything:
                #   P  = (x*scl + biasv) + Mp      AFFINE_THEN_ADD (raw x!)
                #   S  = (P - 2) is_gt At          scalar_tensor_tensor
                #   Mp = (S*-beta + beta) * P      AFFINE_MUL_REDUCE
                Mp = statepool.tile([P, FD], F32, tag="Mp")
                nc.vector.memset(Mp[:], 0.0)
                At0 = statepool.tile([P, FD], F32, tag="At0")
                At1 = statepool.tile([P, FD], F32, tag="At1")
                nc.gpsimd.memset(At0[:], 0.0)
                amr_acc = sv()

                for t in range(T):
                    a_prev, a_next = (At0, At1) if t % 2 == 0 else (At1, At0)
                    if t % SCT == 0:
                        stc = spool.tile([P, SCT * FD], F32, tag="stc")
                    xt = xch[t // CT][:, (t % CT) * FD:(t % CT + 1) * FD]
                    st = stc[:, (t % SCT) * FD:(t % SCT + 1) * FD]
                    pt = ppool.tile([P, FD], F32, tag="pt")
                    nc.vector.affine_then_add(
                        out=pt[:], in0=xt, in1=Mp[:], scale=scl, bias=biasv
                    )
                    nc.vector.scalar_tensor_tensor(
                        out=st, in0=pt[:], scalar=2.0, in1=a_prev[:],
                        op0=AluOpType.subtract, op1=AluOpType.is_gt,
                    )
                    nc.vector.affine_mul_reduce(
                        out=Mp[:], accum_out=amr_acc, in0=st, in1=pt[:],
                        scale=-BETA, bias=BETA,
                    )
                    ad = adpool.tile([P, FD], F32, tag="ad")
                    nc.scalar.mul(ad[:], a_prev[:], DECAY_ADAPT)
                    nc.gpsimd.tensor_tensor(a_next[:], ad[:], st, AluOpType.add)
                    if (t + 1) % SCT == 0:
                        lo = (t + 1 - SCT) % (CT if io_lite else T)
                        nc.scalar.dma_start(
                            out=out_ext[lo:lo + SCT, :, :].rearrange(
                                "t p f -> p t f"
                            ),
                            in_=stc[:].rearrange("p (t f) -> p t f", f=FD),
                        )
    nc.compile()
    return nc


def _shard_host(x, bn_weight, bn_bias):
    """Full [T,B,C,H,W] inputs -> per-core in_maps with device layout."""
    Tn, B, C, H, W = x.shape
    hw = H * W
    nch = C // N_CORES
    x5 = np.asarray(x, dtype=np.float32).reshape(Tn, B, C, hw)
    in_maps = []
    for k in range(N_CORES):
        xs = x5[:, :, k * nch:(k + 1) * nch, :]          # [T,B,nch,hw]
        xp = xs.transpose(0, 2, 1, 3)                    # [T,nch,B,hw]
        xp = xp.reshape(Tn, nch, B // 2, 2, hw)          # b = bh*2+bl
        xp = xp.reshape(Tn, nch * (B // 2), 2 * hw)      # [T,P,FD]
        bw = np.repeat(
            np.asarray(bn_weight[k * nch:(k + 1) * nch], dtype=np.float32), GRP
        ).reshape(P, 1)
        bb = np.repeat(
            np.asarray(bn_bias[k * nch:(k + 1) * nch], dtype=np.float32), GRP
        ).reshape(P, 1)
        in_maps.append(
            {
                "x": np.ascontiguousarray(xp),
                "bnw": np.ascontiguousarray(bw),
                "bnb": np.ascontiguousarray(bb),
            }
        )
    return in_maps


def _unshard_host(outs, T, B, C, H, W):
    """Per-core [T,P,FD] outputs -> full [T,B,C,H,W]."""
    hw = H * W
    nch = C // N_CORES
    parts = []
    for k in range(N_CORES):
        o = outs[k].reshape(T, nch, B // 2, 2, hw)
        o = o.transpose(0, 2, 3, 1, 4).reshape(T, B, nch, H, W)
        parts.append(o)
    return np.concatenate(parts, axis=2).astype(np.float32)


_CACHED = {}


def _get_nc(T, CT, FD, store_group=1, reps=1, io_lite=False, SCT=None,
            deep_bufs=False):
    key = (T, CT, FD, store_group, reps, io_lite, SCT, deep_bufs)
    if key not in _CACHED:
        _CACHED[key] = build_alif(T=T, CT=CT, FD=FD, store_group=store_group,
                                  reps=reps, io_lite=io_lite, SCT=SCT,
                                  deep_bufs=deep_bufs)
    return _CACHED[key]


def run_on_hw(x, bn_weight, bn_bias, trace=False, CT=10, SCT=5, store_group=1, **kwargs):
    T, B, C, H, W = x.shape
    FD = 2 * H * W
    nc = _get_nc(T, CT, FD, store_group, SCT=SCT)
    in_maps = _shard_host(x, bn_weight, bn_bias)
    res = run_bass_kernel_spmd(
        nc, in_maps, core_ids=list(range(N_CORES)), trace=trace, **kwargs
    )
    outs = [np.asarray(r["out"]) for r in res.results]
    full = _unshard_host(outs, T, B, C, H, W)
    return full, res


def kernel(x, bn_weight, bn_bias):
    full, _ = run_on_hw(
        np.asarray(x), np.asarray(bn_weight), np.asarray(bn_bias), trace=False
    )
    return full



# revision 2
# speedup vs baseline: 1.2892x; 1.2892x over previous
"""ALIF (adaptive leaky integrate-and-fire) layer on 8 TRN2 NeuronCores.

Reference semantics (per element over time T):
    BatchNorm3d (training stats over T,B,H,W per channel) then
    mem   = beta*mem + xn_t
    spike = (mem - (1 + adapt) > 0)
    mem   = mem * (1 - spike)
    adapt = decay*adapt + gamma*spike

Sharding: channels C=64 are split 8 ways (8 channels per core) so the BN
batch stats are fully core-local (no collectives needed).  Host pre-lays
each core's shard out as [T, 128, 512] where
    partition p = c_local*16 + (b // 2)      (c_local in 0..7, b in 0..31)
    free      f = (b % 2)*256 + h*16 + w
so the per-channel BN affine is a per-partition scale/bias.  Per-channel
stats while the chunked loads stream in (ScalarE identity-accum sums +
VectorE AFFINE_MUL_REDUCE sumsq), then a 4-round stream_shuffle XOR
butterfly combines each 16-partition group exactly in fp32.

The scan carries 2x-scaled state (scaling fp32 by 2 is exact, making the
threshold a single fused compare) and pre-applies the leak to the stored
membrane.  Two NEW custom DVE ops collapse the whole step onto exactly
three VectorE instructions with NO other engine in the loop:
    DVE  P   = (x*scl + biasv) + Mp          AFFINE_THEN_ADD
    DVE  At' = 0.96*At + ((P-2) > At)        ADAPT_FUSED_ANT (new)
    DVE  Mp  = select((P-2) > At, 0, beta*P) RESET_SELECT_ANT (new)
The spike tensor never exists on device: the adaptive-threshold stream
At' is DMA'd out instead, and the host recovers spikes exactly via
    S_t = (At'_t - fl32(0.96*At'_{t-1}) > 0.5)
which is bit-exact because At' = fl(fl(0.96*At) + S) with S in {0,1} and
the host redoes the same IEEE fp32 multiply.  This removes the
ScalarE/GpSimd adapt path (the old critical cycle S_t -> Pool add 970ns
-> S_{t+1}) entirely: per-step is 3 dependent DVE ops (~1us measured).
"""

import sys

sys.path.insert(0, "/opt/trn_rl_repo")

import numpy as np

import concourse.bass as bass
import concourse.bacc as bacc
import concourse.tile as tile
from concourse import mybir
from concourse.alu_op_type import AluOpType
from concourse.bass_utils import run_bass_kernel_spmd

BETA = 0.9
DECAY_ADAPT = 0.96
BN_EPS = 1e-4

F32 = mybir.dt.float32
AF = mybir.ActivationFunctionType
AX = mybir.AxisListType

N_CORES = 8
P = 128  # SBUF partitions
GRP = 16  # partitions per channel


# --- custom DVE ops (registered at import; rows 17+ are free) -------------- #

def _register_custom_ops():
    import concourse.dve_ops as dve_ops
    from concourse.dve_spec import (
        C0, C2, Spec, Src0, Src1, Zero, _has_src1, lower, select,
    )
    from concourse.dve_uop import DveOpSpec

    def _make(name, spec):
        if name in dve_ops._SUB_OPCODE_FOR_NAME:
            for op in dve_ops.OPS:
                if op.name == name:
                    return op
            raise RuntimeError(f"{name} registered but not in OPS")
        row = dve_ops._CUSTOM_DVE_ROW_BASE + len(dve_ops.OPS)
        assert row < 0x20
        shas = {}
        for ver in ("v3", "v4"):
            tmp = DveOpSpec(
                name=name, opcode=row, uops=lower(spec, ver=ver),
                rd1_en=_has_src1(spec),
            )
            shas[ver] = tmp.sha(ver)
        op = dve_ops.DveOp(name, spec, subdim=False, uops_sha=shas)
        dve_ops.OPS.append(op)
        dve_ops._SUB_OPCODE_FOR_NAME[name] = row
        dve_ops.CUSTOM_DVE_SPECS[name] = spec
        return op

    # At' = s0*At + ((P - imm2) > At);  Src0=P, Src1=At
    adapt = _make(
        "ADAPT_FUSED_ANT",
        Spec(
            body=Src1 * C0 + ((Src0 - C2) > Src1),
            reference=lambda in0, in1, s0, s1, imm2: (
                np.float32(s0) * in1.astype(np.float32)
                + ((in0.astype(np.float32) - np.float32(imm2)) > in1)
            ).astype(np.float32),
        ),
    )
    # Mp = select((P - imm2) > At, 0, s0*P);  Src0=P, Src1=At
    reset = _make(
        "RESET_SELECT_ANT",
        Spec(
            body=select((Src0 - C2) > Src1, Zero, Src0 * C0),
            reference=lambda in0, in1, s0, s1, imm2: np.where(
                (in0.astype(np.float32) - np.float32(imm2)) > in1,
                np.float32(0.0),
                (np.float32(s0) * in0.astype(np.float32)),
            ).astype(np.float32),
        ),
    )
    return adapt, reset


ADAPT_FUSED_ANT, RESET_SELECT_ANT = _register_custom_ops()


def build_alif(T=50, CT=5, FD=512, store_group=1, reps=1, io_lite=False,
               no_scan=False, no_stats=False, SCT=None, deep_bufs=0):
    """Build the single-core Bass graph (SPMD across 8 cores).

    T tiles of [P, FD]; x loaded in chunks of CT tiles.  The DMA'd output
    is the adaptive-threshold stream At' (spikes recovered on host).
    """
    if SCT is None:
        SCT = CT
    assert T % CT == 0 and T % SCT == 0
    nchunks = T // CT
    n_per_channel = float(GRP * T * FD)

    nc = bacc.Bacc("TRN2", target_bir_lowering=False, debug=False)
    # io_lite: identical instruction stream and HBM traffic, but all chunk
    # loads/stores alias one CT-sized DRAM region --- tiny kernel IO so the
    # per-call operand staging doesn't mask device time when benchmarking
    xT = CT if io_lite else T
    x_ext = nc.declare_dram_parameter("x", [xT, P, FD], F32, isOutput=False)
    bnw_ext = nc.declare_dram_parameter("bnw", [P, 1], F32, isOutput=False)
    bnb_ext = nc.declare_dram_parameter("bnb", [P, 1], F32, isOutput=False)
    out_ext = nc.declare_dram_parameter("out", [xT, P, FD], F32, isOutput=True)

    with tile.TileContext(nc) as tc:
        with (
            tc.tile_pool(name="xbuf", bufs=1) as xpool,
            tc.tile_pool(name="stats", bufs=1) as stpool,
            tc.tile_pool(name="state", bufs=1) as statepool,
            tc.tile_pool(name="ptile", bufs=2 + int(deep_bufs)) as ppool,
            tc.tile_pool(name="stile", bufs=2 + int(deep_bufs)) as spool,
        ):
            for _rep in range(reps):
                # scratch tiles for the small stats intermediates
                scr = stpool.tile([P, 48], F32, tag="scratch")
                col = [0]

                def sv(n=1):
                    c = col[0]
                    col[0] += n
                    return scr[:, c:c + n]

                # warm the ScalarE sqrt table set at t=0 so the finalize's
                # real Sqrt doesn't pay the ACT_TABLE_LOAD on the critical path
                warm = sv()
                nc.vector.memset(warm, 1.0)
                wsq = sv()
                nc.scalar.activation(out=wsq, in_=warm, func=AF.Sqrt)

                bnwb = stpool.tile([P, 2], F32, tag="bnwb")
                bnw = bnwb[:, 0:1]
                nc.sync.dma_start(out=bnw, in_=bnw_ext[:, :])
                bnb = bnwb[:, 1:2]
                nc.sync.dma_start(out=bnb, in_=bnb_ext[:, :])

                # ---- load x in chunks; sums chase the DMAs ----
                HC = CT * FD // 2
                s1cols = stpool.tile([P, 2 * nchunks], F32, tag="s1cols")
                s2cols = stpool.tile([P, 2 * nchunks], F32, tag="s2cols")
                dump1 = stpool.tile([P, HC], F32, tag="dump1")
                dump2 = stpool.tile([P, HC], F32, tag="dump2")
                xch = []
                for k in range(nchunks):
                    xk = xpool.tile([P, CT * FD], F32, tag=f"x{k}")
                    nc.sync.dma_start(
                        out=xk[:].rearrange("p (t f) -> p t f", f=FD),
                        in_=x_ext[0 if io_lite else k * CT:
                                  CT if io_lite else (k + 1) * CT, :, :
                                  ].rearrange("t p f -> p t f"),
                    )
                    xch.append(xk)
                    # sync probes: make VectorE/ScalarE observe each chunk's
                    # DMA semaphore early so later ops never carry the wait
                    probe = sv()
                    nc.vector.tensor_copy(probe, xk[:, 0:1])
                    nc.scalar.copy(s1cols[:, 2 * k:2 * k + 1], xk[:, 0:1])
                    if not no_stats:
                        for h in range(2):
                            xh = xk[:, h * HC:(h + 1) * HC]
                            nc.scalar.activation(
                                out=dump1[:], in_=xh, func=AF.Identity,
                                accum_out=s1cols[:, 2 * k + h:2 * k + h + 1],
                            )
                            nc.vector.affine_mul_reduce(
                                out=dump2[:], in0=xh, in1=xh,
                                scale=1.0, bias=0.0,
                                accum_out=s2cols[:, 2 * k + h:2 * k + h + 1],
                            )

                # ---- finalize BN stats -> per-partition scale/bias ----
                if no_stats:
                    scl, biasv = 1.0, 0.0
                else:
                    E = sv(2)
                    nc.vector.tensor_reduce(
                        out=E[:, 0:1], in_=s1cols[:], axis=AX.X, op=AluOpType.add
                    )
                    nc.vector.tensor_reduce(
                        out=E[:, 1:2], in_=s2cols[:], axis=AX.X, op=AluOpType.add
                    )
                    cur = E
                    for i, k in enumerate((8, 4, 2, 1)):
                        sh = sv(2)
                        nc.vector.stream_shuffle(sh, cur, [j ^ k for j in range(32)])
                        nxt = sv(2)
                        nc.vector.tensor_tensor(nxt, cur, sh, AluOpType.add)
                        cur = nxt

                    me = sv(2)
                    nc.vector.tensor_scalar_mul(me, cur[:, 0:2], 1.0 / n_per_channel)
                    mean = me[:, 0:1]
                    ex2 = me[:, 1:2]
                    msq = sv()
                    nc.vector.tensor_tensor(msq, mean, mean, AluOpType.mult)
                    var = sv()
                    nc.vector.tensor_tensor(var, ex2, msq, AluOpType.subtract)
                    vpe = sv()
                    nc.vector.tensor_scalar_add(vpe, var, BN_EPS)

                    # rstd = rsqrt(vpe): ACT sqrt seed + DVE recip + 2 Newton
                    sq = sv()
                    nc.scalar.activation(out=sq, in_=vpe, func=AF.Sqrt)
                    ya = sv()
                    nc.vector.reciprocal(ya, sq)
                    yy = sv()
                    u = sv()
                    w = sv()
                    yb = sv()
                    yc = sv()
                    ys = [ya, yb, yc]
                    for i in range(2):
                        nc.vector.tensor_tensor(yy, ys[i], ys[i], AluOpType.mult)
                        nc.vector.tensor_tensor(u, yy, vpe, AluOpType.mult)
                        nc.vector.tensor_scalar(
                            out=w, in0=u, scalar1=-0.5, scalar2=1.5,
                            op0=AluOpType.mult, op1=AluOpType.add,
                        )
                        nc.vector.tensor_tensor(ys[i + 1], ys[i], w, AluOpType.mult)
                    rstd = ys[2]

                    # scale = 2*bnw*rstd ; bias = 2*bnb - mean*scale
                    scl = sv()
                    nc.vector.scalar_tensor_tensor(
                        out=scl, in0=bnw, scalar=2.0, in1=rstd,
                        op0=AluOpType.mult, op1=AluOpType.mult,
                    )
                    tmpb = sv()
                    nc.vector.tensor_tensor(tmpb, mean, scl, AluOpType.mult)
                    biasv = sv()
                    nc.vector.scalar_tensor_tensor(
                        out=biasv, in0=bnb, scalar=2.0, in1=tmpb,
                        op0=AluOpType.mult, op1=AluOpType.subtract,
                    )

                if no_scan:
                    nc.sync.dma_start(out=out_ext[0:1, :, 0:4],
                                      in_=x_ext[0:1, :, 0:4])
                    continue
                # ---- the scan: 3 dependent DVE ops per step ----
                #   P   = (x*scl + biasv) + Mp       AFFINE_THEN_ADD
                #   At' = 0.96*At + ((P-2) > At)     ADAPT_FUSED_ANT
                #   Mp  = select((P-2) > At, 0, b*P) RESET_SELECT_ANT
                # At' tiles double as the DMA'd output (host recovers spikes).
                Mp = statepool.tile([P, FD], F32, tag="Mp")
                nc.vector.memset(Mp[:], 0.0)
                at0 = statepool.tile([P, FD], F32, tag="at0")
                nc.vector.memset(at0[:], 0.0)
                at_prev = at0[:]

                for t in range(T):
                    if t % SCT == 0:
                        atc = spool.tile([P, SCT * FD], F32, tag="atc")
                    at_cur = atc[:, (t % SCT) * FD:(t % SCT + 1) * FD]
                    xt = xch[t // CT][:, (t % CT) * FD:(t % CT + 1) * FD]
                    pt = ppool.tile([P, FD], F32, tag="pt")
                    nc.vector.affine_then_add(
                        out=pt[:], in0=xt, in1=Mp[:], scale=scl, bias=biasv
                    )
                    nc.vector._custom_dve(
                        ADAPT_FUSED_ANT, out=at_cur, in0=pt[:], in1=at_prev,
                        s0=DECAY_ADAPT, imm2=2.0,
                    )
                    nc.vector._custom_dve(
                        RESET_SELECT_ANT, out=Mp[:], in0=pt[:], in1=at_prev,
                        s0=BETA, imm2=2.0,
                    )
                    at_prev = at_cur
                    if (t + 1) % SCT == 0:
                        lo = (t + 1 - SCT) % (CT if io_lite else T)
                        nc.scalar.dma_start(
                            out=out_ext[lo:lo + SCT, :, :].rearrange(
                                "t p f -> p t f"
                            ),
                            in_=atc[:].rearrange("p (t f) -> p t f", f=FD),
                        )
    nc.compile()
    return nc


def _shard_host(x, bn_weight, bn_bias):
    """Full [T,B,C,H,W] inputs -> per-core in_maps with device layout."""
    Tn, B, C, H, W = x.shape
    hw = H * W
    nch = C // N_CORES
    x5 = np.asarray(x, dtype=np.float32).reshape(Tn, B, C, hw)
    in_maps = []
    for k in range(N_CORES):
        xs = x5[:, :, k * nch:(k + 1) * nch, :]          # [T,B,nch,hw]
        xp = xs.transpose(0, 2, 1, 3)                    # [T,nch,B,hw]
        xp = xp.reshape(Tn, nch, B // 2, 2, hw)          # b = bh*2+bl
        xp = xp.reshape(Tn, nch * (B // 2), 2 * hw)      # [T,P,FD]
        bw = np.repeat(
            np.asarray(bn_weight[k * nch:(k + 1) * nch], dtype=np.float32), GRP
        ).reshape(P, 1)
        bb = np.repeat(
            np.asarray(bn_bias[k * nch:(k + 1) * nch], dtype=np.float32), GRP
        ).reshape(P, 1)
        in_maps.append(
            {
                "x": np.ascontiguousarray(xp),
                "bnw": np.ascontiguousarray(bw),
                "bnb": np.ascontiguousarray(bb),
            }
        )
    return in_maps


def _recover_spikes(at):
    """[T,P,FD] At' stream -> spike tensor, exactly.

    At'_t = fl(fl(0.96*At_{t-1}) + S_t) with S in {0,1}: redo the fp32
    multiply and threshold the difference at 0.5."""
    prev = np.empty_like(at)
    prev[0] = 0.0
    prev[1:] = at[:-1]
    dec = (np.float32(DECAY_ADAPT) * prev).astype(np.float32)
    return ((at - dec) > np.float32(0.5)).astype(np.float32)


def _unshard_host(outs, T, B, C, H, W):
    """Per-core [T,P,FD] At' outputs -> full [T,B,C,H,W] spikes."""
    hw = H * W
    nch = C // N_CORES
    parts = []
    for k in range(N_CORES):
        s = _recover_spikes(np.asarray(outs[k]))
        o = s.reshape(T, nch, B // 2, 2, hw)
        o = o.transpose(0, 2, 3, 1, 4).reshape(T, B, nch, H, W)
        parts.append(o)
    return np.concatenate(parts, axis=2).astype(np.float32)


_CACHED = {}


def _get_nc(T, CT, FD, store_group=1, reps=1, io_lite=False, SCT=None,
            deep_bufs=False):
    key = (T, CT, FD, store_group, reps, io_lite, SCT, deep_bufs)
    if key not in _CACHED:
        _CACHED[key] = build_alif(T=T, CT=CT, FD=FD, store_group=store_group,
                                  reps=reps, io_lite=io_lite, SCT=SCT,
                                  deep_bufs=deep_bufs)
    return _CACHED[key]


def run_on_hw(x, bn_weight, bn_bias, trace=False, CT=10, SCT=5, store_group=1, **kwargs):
    T, B, C, H, W = x.shape
    FD = 2 * H * W
    nc = _get_nc(T, CT, FD, store_group, SCT=SCT)
    in_maps = _shard_host(x, bn_weight, bn_bias)
    res = run_bass_kernel_spmd(
        nc, in_maps, core_ids=list(range(N_CORES)), trace=trace, **kwargs
    )
    outs = [np.asarray(r["out"]) for r in res.results]
    full = _unshard_host(outs, T, B, C, H, W)
    return full, res


def kernel(x, bn_weight, bn_bias):
    full, _ = run_on_hw(
        np.asarray(x), np.asarray(bn_weight), np.asarray(bn_bias), trace=False
    )
    return full


# revision 8
# speedup vs baseline: 1.8635x; 1.4455x over previous
"""ALIF (adaptive leaky integrate-and-fire) layer on 8 TRN2 NeuronCores.

Reference semantics (per element over time T):
    BatchNorm3d (training stats over T,B,H,W per channel) then
    mem   = beta*mem + xn_t
    spike = (mem - (1 + adapt) > 0)
    mem   = mem * (1 - spike)
    adapt = decay*adapt + gamma*spike

Sharding: channels C=64 are split 8 ways (8 channels per core) so the BN
batch stats are fully core-local (no collectives needed).  Host pre-lays
each core's shard out as [T, 128, 512] where
    partition p = c_local*16 + (b // 2)      (c_local in 0..7, b in 0..31)
    free      f = (b % 2)*256 + h*16 + w
so the per-channel BN affine is a per-partition scale/bias.  Per-channel
stats while the chunked loads stream in (ScalarE identity-accum sums +
VectorE AFFINE_MUL_REDUCE sumsq), then a 4-round stream_shuffle XOR
butterfly combines each 16-partition group exactly in fp32.

The scan carries 2x-scaled state (scaling fp32 by 2 is exact, making the
threshold a single fused compare) and pre-applies the leak to the stored
membrane.  Two NEW custom DVE ops collapse the whole step onto exactly
three VectorE instructions with NO other engine in the loop:
    DVE  P   = (x*scl + biasv) + Mp          AFFINE_THEN_ADD
    DVE  At' = 0.96*At + ((P-2) > At)        ADAPT_FUSED_ANT (new)
    DVE  Mp  = select((P-2) > At, 0, beta*P) RESET_SELECT_ANT (new)
The spike tensor never exists on device: the adaptive-threshold stream
At' is DMA'd out instead, and the host recovers spikes exactly via
    S_t = (At'_t - fl32(0.96*At'_{t-1}) > 0.5)
which is bit-exact because At' = fl(fl(0.96*At) + S) with S in {0,1} and
the host redoes the same IEEE fp32 multiply.  This removes the
ScalarE/GpSimd adapt path (the old critical cycle S_t -> Pool add 970ns
-> S_{t+1}) entirely: per-step is 3 dependent DVE ops (~1us measured).
"""

import sys

sys.path.insert(0, "/opt/trn_rl_repo")

import numpy as np

import concourse.bass as bass
import concourse.bacc as bacc
import concourse.tile as tile
from concourse import mybir
from concourse.alu_op_type import AluOpType
from concourse.bass_utils import run_bass_kernel_spmd

BETA = 0.9
DECAY_ADAPT = 0.96
BN_EPS = 1e-4

F32 = mybir.dt.float32
AF = mybir.ActivationFunctionType
AX = mybir.AxisListType

N_CORES = 8
P = 128  # SBUF partitions
GRP = 16  # partitions per channel


# --- custom DVE ops (registered at import; rows 17+ are free) -------------- #

def _register_custom_ops():
    import concourse.dve_ops as dve_ops
    from concourse.dve_spec import (
        C0, C2, Spec, Src0, Src1, Zero, _has_src1, lower, select,
    )
    from concourse.dve_uop import DveOpSpec

    def _make(name, spec):
        if name in dve_ops._SUB_OPCODE_FOR_NAME:
            for op in dve_ops.OPS:
                if op.name == name:
                    return op
            raise RuntimeError(f"{name} registered but not in OPS")
        row = dve_ops._CUSTOM_DVE_ROW_BASE + len(dve_ops.OPS)
        assert row < 0x20
        shas = {}
        for ver in ("v3", "v4"):
            tmp = DveOpSpec(
                name=name, opcode=row, uops=lower(spec, ver=ver),
                rd1_en=_has_src1(spec),
            )
            shas[ver] = tmp.sha(ver)
        op = dve_ops.DveOp(name, spec, subdim=False, uops_sha=shas)
        dve_ops.OPS.append(op)
        dve_ops._SUB_OPCODE_FOR_NAME[name] = row
        dve_ops.CUSTOM_DVE_SPECS[name] = spec
        return op

    # At' = s0*At + ((P - imm2) > At);  Src0=P, Src1=At
    adapt = _make(
        "ADAPT_FUSED_ANT",
        Spec(
            body=Src1 * C0 + ((Src0 - C2) > Src1),
            reference=lambda in0, in1, s0, s1, imm2: (
                np.float32(s0) * in1.astype(np.float32)
                + ((in0.astype(np.float32) - np.float32(imm2)) > in1)
            ).astype(np.float32),
        ),
    )
    # Mp = select((P - imm2) > At, 0, s0*P);  Src0=P, Src1=At
    reset = _make(
        "RESET_SELECT_ANT",
        Spec(
            body=select((Src0 - C2) > Src1, Zero, Src0 * C0),
            reference=lambda in0, in1, s0, s1, imm2: np.where(
                (in0.astype(np.float32) - np.float32(imm2)) > in1,
                np.float32(0.0),
                (np.float32(s0) * in0.astype(np.float32)),
            ).astype(np.float32),
        ),
    )
    return adapt, reset


ADAPT_FUSED_ANT, RESET_SELECT_ANT = _register_custom_ops()


def build_alif(T=50, CT=5, FD=512, store_group=1, reps=1, io_lite=False,
               no_scan=False, no_stats=False, SCT=None, deep_bufs=0,
               scan_mult=1, stats_mult=1, float_affine=False,
               no_outdma=False):
    """Build the single-core Bass graph (SPMD across 8 cores).

    T tiles of [P, FD]; x loaded in chunks of CT tiles.  The DMA'd output
    is the adaptive-threshold stream At' (spikes recovered on host).
    """
    if SCT is None:
        SCT = CT
    assert T % CT == 0 and T % SCT == 0
    nchunks = T // CT
    n_per_channel = float(GRP * T * FD)

    nc = bacc.Bacc("TRN2", target_bir_lowering=False, debug=False)
    # io_lite: identical instruction stream and HBM traffic, but all chunk
    # loads/stores alias one CT-sized DRAM region --- tiny kernel IO so the
    # per-call operand staging doesn't mask device time when benchmarking
    xT = CT if io_lite else T
    x_ext = nc.declare_dram_parameter("x", [xT, P, FD], F32, isOutput=False)
    bnw_ext = nc.declare_dram_parameter("bnw", [P, 1], F32, isOutput=False)
    bnb_ext = nc.declare_dram_parameter("bnb", [P, 1], F32, isOutput=False)
    out_ext = nc.declare_dram_parameter("out", [xT, P, FD], F32, isOutput=True)

    with tile.TileContext(nc) as tc:
        with (
            tc.tile_pool(name="xbuf", bufs=1) as xpool,
            tc.tile_pool(name="stats", bufs=1) as stpool,
            tc.tile_pool(name="state", bufs=1) as statepool,
            tc.tile_pool(name="ptile", bufs=3 + int(deep_bufs)) as ppool,
            tc.tile_pool(name="xn", bufs=3 + int(deep_bufs)) as xnpool,
            tc.tile_pool(name="stile", bufs=4 + int(deep_bufs)) as spool,
        ):
            for _rep in range(reps):
                # scratch tiles for the small stats intermediates
                scr = stpool.tile([P, 48], F32, tag="scratch")
                col = [0]

                def sv(n=1):
                    c = col[0]
                    col[0] += n
                    return scr[:, c:c + n]

                # warm the ScalarE sqrt table set at t=0 so the finalize's
                # real Sqrt doesn't pay the ACT_TABLE_LOAD on the critical path
                warm = sv()
                nc.vector.memset(warm, 1.0)
                wsq = sv()
                nc.scalar.activation(out=wsq, in_=warm, func=AF.Sqrt)

                bnwb = stpool.tile([P, 2], F32, tag="bnwb")
                bnw = bnwb[:, 0:1]
                nc.sync.dma_start(out=bnw, in_=bnw_ext[:, :])
                bnb = bnwb[:, 1:2]
                nc.sync.dma_start(out=bnb, in_=bnb_ext[:, :])

                # ---- load x in chunks; sums chase the DMAs ----
                HC = CT * FD // 2
                s1cols = stpool.tile([P, 2 * nchunks], F32, tag="s1cols")
                s2cols = stpool.tile([P, 2 * nchunks], F32, tag="s2cols")
                dump1 = stpool.tile([P, HC], F32, tag="dump1")
                dump2 = stpool.tile([P, HC], F32, tag="dump2")
                xch = []
                for k in range(nchunks):
                    xk = xpool.tile([P, CT * FD], F32, tag=f"x{k}")
                    nc.sync.dma_start(
                        out=xk[:].rearrange("p (t f) -> p t f", f=FD),
                        in_=x_ext[0 if io_lite else k * CT:
                                  CT if io_lite else (k + 1) * CT, :, :
                                  ].rearrange("t p f -> p t f"),
                    )
                    xch.append(xk)
                    # sync probes: make VectorE/ScalarE observe each chunk's
                    # DMA semaphore early so later ops never carry the wait
                    probe = sv()
                    nc.vector.tensor_copy(probe, xk[:, 0:1])
                    nc.scalar.copy(s1cols[:, 2 * k:2 * k + 1], xk[:, 0:1])
                    if not no_stats:
                        for _sm in range(stats_mult):
                            xh0 = xk[:, 0:HC]
                            xh1 = xk[:, HC:2 * HC]
                            # h=0: ACT sum + DVE sumsq; h=1: POOL sum + ACT
                            # sumsq (Square is bit-exact, see microbench)
                            nc.scalar.activation(
                                out=dump1[:], in_=xh0, func=AF.Identity,
                                accum_out=s1cols[:, 2 * k:2 * k + 1],
                            )
                            nc.vector.affine_mul_reduce(
                                out=dump2[:], in0=xh0, in1=xh0,
                                scale=1.0, bias=0.0,
                                accum_out=s2cols[:, 2 * k:2 * k + 1],
                            )
                            nc.scalar.activation(
                                out=dump1[:], in_=xh1, func=AF.Identity,
                                accum_out=s1cols[:, 2 * k + 1:2 * k + 2],
                            )
                            nc.scalar.activation(
                                out=dump1[:], in_=xh1, func=AF.Square,
                                accum_out=s2cols[:, 2 * k + 1:2 * k + 2],
                            )

                # ---- finalize BN stats -> per-partition scale/bias ----
                if no_stats:
                    scl, biasv = 1.0, 0.0
                else:
                    E = sv(2)
                    nc.vector.tensor_reduce(
                        out=E[:, 0:1], in_=s1cols[:], axis=AX.X, op=AluOpType.add
                    )
                    nc.vector.tensor_reduce(
                        out=E[:, 1:2], in_=s2cols[:], axis=AX.X, op=AluOpType.add
                    )
                    cur = E
                    for i, k in enumerate((8, 4, 2, 1)):
                        sh = sv(2)
                        nc.vector.stream_shuffle(sh, cur, [j ^ k for j in range(32)])
                        nxt = sv(2)
                        nc.vector.tensor_tensor(nxt, cur, sh, AluOpType.add)
                        cur = nxt

                    me = sv(2)
                    nc.vector.tensor_scalar_mul(me, cur[:, 0:2], 1.0 / n_per_channel)
                    mean = me[:, 0:1]
                    ex2 = me[:, 1:2]
                    msq = sv()
                    nc.vector.tensor_tensor(msq, mean, mean, AluOpType.mult)
                    var = sv()
                    nc.vector.tensor_tensor(var, ex2, msq, AluOpType.subtract)
                    vpe = sv()
                    nc.vector.tensor_scalar_add(vpe, var, BN_EPS)

                    # rstd = rsqrt(vpe): ACT sqrt seed + DVE recip + 2 Newton
                    sq = sv()
                    nc.scalar.activation(out=sq, in_=vpe, func=AF.Sqrt)
                    ya = sv()
                    nc.vector.reciprocal(ya, sq)
                    yy = sv()
                    u = sv()
                    w = sv()
                    yb = sv()
                    yc = sv()
                    ys = [ya, yb, yc]
                    for i in range(2):
                        nc.vector.tensor_tensor(yy, ys[i], ys[i], AluOpType.mult)
                        nc.vector.tensor_tensor(u, yy, vpe, AluOpType.mult)
                        nc.vector.tensor_scalar(
                            out=w, in0=u, scalar1=-0.5, scalar2=1.5,
                            op0=AluOpType.mult, op1=AluOpType.add,
                        )
                        nc.vector.tensor_tensor(ys[i + 1], ys[i], w, AluOpType.mult)
                    rstd = ys[2]

                    # scale = 2*bnw*rstd ; bias = 2*bnb - mean*scale
                    scl = sv()
                    nc.vector.scalar_tensor_tensor(
                        out=scl, in0=bnw, scalar=2.0, in1=rstd,
                        op0=AluOpType.mult, op1=AluOpType.mult,
                    )
                    tmpb = sv()
                    nc.vector.tensor_tensor(tmpb, mean, scl, AluOpType.mult)
                    biasv = sv()
                    nc.vector.scalar_tensor_tensor(
                        out=biasv, in0=bnb, scalar=2.0, in1=tmpb,
                        op0=AluOpType.mult, op1=AluOpType.subtract,
                    )

                if no_scan or float_affine:
                    # keep the stats chain (and so the loads) live
                    nc.scalar.dma_start(
                        out=out_ext[0:1, :, 0:48].rearrange("t p f -> p t f"),
                        in_=scr[:].rearrange("p (t f) -> p t f", f=48),
                    )
                    if no_scan:
                        continue
                if float_affine:
                    scl, biasv = 1.0, 0.0
                # ---- the scan: 3 dependent DVE ops per step ----
                #   P   = (x*scl + biasv) + Mp       AFFINE_THEN_ADD
                #   At' = 0.96*At + ((P-2) > At)     ADAPT_FUSED_ANT
                #   Mp  = select((P-2) > At, 0, b*P) RESET_SELECT_ANT
                # At' tiles double as the DMA'd output (host recovers spikes).
                for _sc in range(scan_mult):
                    Mp = statepool.tile([P, FD], F32, tag="Mp")
                    nc.vector.memset(Mp[:], 0.0)
                    at0 = statepool.tile([P, FD], F32, tag="at0")
                    nc.vector.memset(at0[:], 0.0)
                    at_prev = at0[:]

                    for t in range(T):
                        if t % SCT == 0:
                            atc = spool.tile([P, SCT * FD], F32, tag="atc")
                        at_cur = atc[:, (t % SCT) * FD:(t % SCT + 1) * FD]
                        xt = xch[t // CT][:, (t % CT) * FD:(t % CT + 1) * FD]
                        pt = ppool.tile([P, FD], F32, tag="pt")
                        if float_affine:
                            nc.vector.affine_then_add(
                                out=pt[:], in0=xt, in1=Mp[:], scale=scl,
                                bias=biasv,
                            )
                        else:
                            # ACT applies the BN affine off the critical path
                            # (bit-exact: per-stage fp32 rounding matches ata)
                            xn = xnpool.tile([P, FD], F32, tag="xn")
                            nc.scalar.activation(
                                out=xn[:], in_=xt, func=AF.Identity,
                                scale=scl, bias=biasv,
                            )
                            nc.vector.tensor_tensor(
                                pt[:], xn[:], Mp[:], AluOpType.add
                            )
                        nc.vector._custom_dve(
                            ADAPT_FUSED_ANT, out=at_cur, in0=pt[:],
                            in1=at_prev, s0=DECAY_ADAPT, imm2=2.0,
                        )
                        nc.vector._custom_dve(
                            RESET_SELECT_ANT, out=Mp[:], in0=pt[:],
                            in1=at_prev, s0=BETA, imm2=2.0,
                        )
                        at_prev = at_cur
                        if (t + 1) % SCT == 0 and (
                                not no_outdma or t + 1 == T):
                            lo = (t + 1 - SCT) % (CT if io_lite else T)
                            nc.scalar.dma_start(
                                out=out_ext[lo:lo + SCT, :, :].rearrange(
                                    "t p f -> p t f"
                                ),
                                in_=atc[:].rearrange("p (t f) -> p t f", f=FD),
                            )
    nc.compile()
    return nc


def _shard_host(x, bn_weight, bn_bias):
    """Full [T,B,C,H,W] inputs -> per-core in_maps with device layout."""
    Tn, B, C, H, W = x.shape
    hw = H * W
    nch = C // N_CORES
    x5 = np.asarray(x, dtype=np.float32).reshape(Tn, B, C, hw)
    in_maps = []
    for k in range(N_CORES):
        xs = x5[:, :, k * nch:(k + 1) * nch, :]          # [T,B,nch,hw]
        xp = xs.transpose(0, 2, 1, 3)                    # [T,nch,B,hw]
        xp = xp.reshape(Tn, nch, B // 2, 2, hw)          # b = bh*2+bl
        xp = xp.reshape(Tn, nch * (B // 2), 2 * hw)      # [T,P,FD]
        bw = np.repeat(
            np.asarray(bn_weight[k * nch:(k + 1) * nch], dtype=np.float32), GRP
        ).reshape(P, 1)
        bb = np.repeat(
            np.asarray(bn_bias[k * nch:(k + 1) * nch], dtype=np.float32), GRP
        ).reshape(P, 1)
        in_maps.append(
            {
                "x": np.ascontiguousarray(xp),
                "bnw": np.ascontiguousarray(bw),
                "bnb": np.ascontiguousarray(bb),
            }
        )
    return in_maps


def _recover_spikes(at):
    """[T,P,FD] At' stream -> spike tensor, exactly.

    At'_t = fl(fl(0.96*At_{t-1}) + S_t) with S in {0,1}: redo the fp32
    multiply and threshold the difference at 0.5."""
    prev = np.empty_like(at)
    prev[0] = 0.0
    prev[1:] = at[:-1]
    dec = (np.float32(DECAY_ADAPT) * prev).astype(np.float32)
    return ((at - dec) > np.float32(0.5)).astype(np.float32)


def _unshard_host(outs, T, B, C, H, W):
    """Per-core [T,P,FD] At' outputs -> full [T,B,C,H,W] spikes."""
    hw = H * W
    nch = C // N_CORES
    parts = []
    for k in range(N_CORES):
        s = _recover_spikes(np.asarray(outs[k]))
        o = s.reshape(T, nch, B // 2, 2, hw)
        o = o.transpose(0, 2, 3, 1, 4).reshape(T, B, nch, H, W)
        parts.append(o)
    return np.concatenate(parts, axis=2).astype(np.float32)


_CACHED = {}


def _get_nc(T, CT, FD, store_group=1, reps=1, io_lite=False, SCT=None,
            deep_bufs=False, **kw):
    key = (T, CT, FD, store_group, reps, io_lite, SCT, deep_bufs,
           tuple(sorted(kw.items())))
    if key not in _CACHED:
        _CACHED[key] = build_alif(T=T, CT=CT, FD=FD, store_group=store_group,
                                  reps=reps, io_lite=io_lite, SCT=SCT,
                                  deep_bufs=deep_bufs, **kw)
    return _CACHED[key]


def run_on_hw(x, bn_weight, bn_bias, trace=False, CT=10, SCT=5, store_group=1, **kwargs):
    T, B, C, H, W = x.shape
    FD = 2 * H * W
    nc = _get_nc(T, CT, FD, store_group, SCT=SCT)
    in_maps = _shard_host(x, bn_weight, bn_bias)
    res = run_bass_kernel_spmd(
        nc, in_maps, core_ids=list(range(N_CORES)), trace=trace, **kwargs
    )
    outs = [np.asarray(r["out"]) for r in res.results]
    full = _unshard_host(outs, T, B, C, H, W)
    return full, res


def kernel(x, bn_weight, bn_bias):
    full, _ = run_on_hw(
        np.asarray(x), np.asarray(bn_weight), np.asarray(bn_bias), trace=False
    )
    return full
